# revision 1
# baseline (speedup 1.0000x reference)
"""MoE layer (SwiGLU experts, top-2 routing) on 8 Trainium2 NeuronCores.

Strategy (expert parallelism, per the sharding hint):
  - The router (a [N,8] matmul + softmax + top-2, ~0.01% of total FLOPs) is
    computed host-side in float64; it determines the token->expert dispatch.
  - Token dispatch/combine (the "all-to-all") is done host-side: each core e
    receives expert e's weights plus the tokens routed to expert e, padded to
    a uniform capacity C (multiple of 128, same on all cores for SPMD).
  - Each core runs the heavy compute: y = (silu(x@wg) * (x@wu)) @ wd scaled
    by the per-token combine weight, all matmuls in float32r (full PE rate).
  - Host scatter-adds each expert's output rows back into the final output.

Device kernel structure (per core):
  Tokens are processed in groups of up to 768. Stage 1 computes
  hT[f, token] = silu(wg.T x) * (wu.T x) for all F=4096 rows of the group,
  accumulating over D=1024 in PSUM (8 matmuls per 128-row f-tile), with the
  gate/up PSUM banks drained by ScalarE (silu) and VectorE (mul) into SBUF.
  Stage 2 contracts hT over F entirely in PSUM (32-matmul accumulation per
  output tile), applies the combine weight, and streams results out.
"""

import os
import sys

sys.path.insert(0, "/opt/trn_rl_repo")
import numpy as np

P = 128
D_MODEL = 1024
D_FF = 4096
N_EXPERTS = 8
TOP_K = 2
G_MAX = 768  # token group size; stage-2 runs one 6-wide batch per d0 so
# wd streams exactly once per group (~190 GB/s, under the HBM wall)

LAST_EXEC_NS = None
_programs = {}


def _ensure_axon_hooks():
    """The agent image's antenv lacks axon_hooks; reconstruct it so
    trace=True works (NTFF profiling via libaxon_pjrt ctypes hook)."""
    import types

    try:
        import antenv.axon_hooks  # noqa: F401

        return
    except ImportError:
        pass
    try:
        import antenv

        mod = types.ModuleType("antenv.axon_hooks")
        _hook = [None]
        mod.set_axon_ntff_profile_hook = lambda h: _hook.__setitem__(0, h)
        mod.get_axon_ntff_profile_hook = lambda: _hook[0]
        sys.modules["antenv.axon_hooks"] = mod
        antenv.axon_hooks = mod
        if "/root/.axon_site" not in sys.path:
            sys.path.insert(0, "/root/.axon_site")
        from trn_agent_boot.trn_boot import _ntff_profile_via_ctypes

        mod.set_axon_ntff_profile_hook(
            _ntff_profile_via_ctypes("/opt/axon/libaxon_pjrt.so")
        )
        import concourse.bass_utils as bu

        bu.upload_artifacts = lambda tmpdir: f"local://{tmpdir}"
    except Exception:
        pass


def _build_program(C):
    import concourse.bacc as bacc
    import concourse.mybir as mybir
    from concourse.tile import TileContext

    fp32 = mybir.dt.float32
    f32r = mybir.dt.float32r
    D, F = D_MODEL, D_FF
    DT, FT = D // P, F // P
    silu_fn = mybir.ActivationFunctionType.Silu
    mult_op = mybir.AluOpType.mult

    nc = bacc.Bacc(
        "TRN2", target_bir_lowering=False, debug=False, num_devices=N_EXPERTS
    )
    xT = nc.dram_tensor("xT", [D, C], fp32, kind="ExternalInput")
    wg = nc.dram_tensor("wg", [D, F], fp32, kind="ExternalInput")
    wu = nc.dram_tensor("wu", [D, F], fp32, kind="ExternalInput")
    wd = nc.dram_tensor("wd", [F, D], fp32, kind="ExternalInput")
    sc = nc.dram_tensor("sc", [C], fp32, kind="ExternalInput")
    y = nc.dram_tensor("y", [C, D], fp32, kind="ExternalOutput")

    xT_r = xT.ap().bitcast(f32r).rearrange("(dt p) c -> p dt c", p=P)
    wg_r = wg.ap().bitcast(f32r).rearrange("(dt p) f -> p dt f", p=P)
    wu_r = wu.ap().bitcast(f32r).rearrange("(dt p) f -> p dt f", p=P)
    wd_r = wd.ap().bitcast(f32r).rearrange("(ft p) d -> p ft d", p=P)
    sc_r = sc.ap().rearrange("(g p) -> p g", p=P)
    y_ap = y.ap()

    # split C into groups of <=768, avoiding tail groups <512 (supply-bound)
    sizes = []
    rem = C
    while rem > 0:
        if rem >= G_MAX + 512 or rem <= G_MAX:
            take = min(G_MAX, rem)
        else:
            take = rem - 512  # leaves a 512 tail; keeps this group >=512
        sizes.append(take)
        rem -= take
    groups = []
    g0 = 0
    for gc in sizes:
        groups.append((g0, gc))
        g0 += gc

    def batch_plan(gsub):
        # one batch if it fits the 6 stage-2 PSUM banks, else split evenly
        if gsub <= 6:
            return [list(range(gsub))]
        n_b = (gsub + 5) // 6
        base, extra = divmod(gsub, n_b)
        out, s = [], 0
        for i in range(n_b):
            n = base + (1 if i < extra else 0)
            out.append(list(range(s, s + n)))
            s += n
        return out

    with TileContext(nc) as tc:
        with (
            tc.tile_pool(name="warm", bufs=1) as warm_pool,
            tc.tile_pool(name="xg", bufs=2) as xg_pool,
            tc.tile_pool(name="wgu", bufs=4) as wgu_pool,
            tc.tile_pool(name="ht", bufs=FT) as ht_pool,
            tc.tile_pool(name="wdp", bufs=5) as wd_pool,
            tc.tile_pool(name="act", bufs=2) as act_pool,
            tc.tile_pool(name="out", bufs=2) as out_pool,
            tc.tile_pool(name="scp", bufs=2) as sc_pool,
            tc.tile_pool(name="ps1", bufs=1, space="PSUM") as ps1_pool,
            tc.tile_pool(name="ps2", bufs=6, space="PSUM") as ps2_pool,
        ):
            # Warm-up: keep TensorE busy while the first tiles stream in, so
            # the HAM clock gate reaches 2.4 GHz before real matmuls start.
            wsrc = warm_pool.tile([P, 256], f32r, name="wsrc")
            nc.vector.memset(wsrc.bitcast(fp32)[:], 0.0)
            wps = ps1_pool.tile([P, 512], fp32, name="psg")
            for wi in range(60):
                nc.tensor.matmul(
                    wps[:, :256],
                    wsrc[:, :P],
                    wsrc[:],
                    start=(wi == 0),
                    stop=(wi == 59),
                )
            for g0, gc in groups:
                gsub = gc // P
                batches = batch_plan(gsub)

                # hoist the first f-tile's weights ahead of the token slab so
                # stage 1 can start as soon as xg's first slice lands
                wgu_pre = {}
                for ft in range(1):
                    wgt = wgu_pool.tile([P, DT, P], f32r, name="wgt")
                    nc.sync.dma_start(
                        out=wgt[:], in_=wg_r[:, :, ft * P : (ft + 1) * P]
                    )
                    wut = wgu_pool.tile([P, DT, P], f32r, name="wut")
                    nc.sync.dma_start(
                        out=wut[:], in_=wu_r[:, :, ft * P : (ft + 1) * P]
                    )
                    wgu_pre[ft] = (wgt, wut)

                xg = xg_pool.tile([P, DT, G_MAX], f32r, name="xg")
                # per-dt loads so stage-1 matmuls can start on the first slice
                for dt_i in range(DT):
                    nc.sync.dma_start(
                        out=xg[:, dt_i, :gc], in_=xT_r[:, dt_i, g0 : g0 + gc]
                    )
                sct = sc_pool.tile([P, G_MAX // P], fp32, name="sct")
                nc.gpsimd.dma_start(
                    out=sct[:, :gsub], in_=sc_r[:, g0 // P : g0 // P + gsub]
                )

                # stage-2 wd load schedule; hoist the first few DMAs so the
                # scalar queue primes the pipeline during stage 1
                wd_sched = [
                    (d0, bi, ft0)
                    for d0 in range(0, D_MODEL, 512)
                    for bi in range(len(batches))
                    for ft0 in range(0, FT, 2)
                ]
                wd_pre = {}
                for key in wd_sched[:5]:
                    d0, bi, ft0 = key
                    wdt = wd_pool.tile([P, 2, 512], f32r, name="wdt")
                    nc.scalar.dma_start(
                        out=wdt[:], in_=wd_r[:, ft0 : ft0 + 2, d0 : d0 + 512]
                    )
                    wd_pre[key] = wdt

                # equal-width chunks: small-N matmuls cost ~140ns regardless of
                # width, so [384,384] beats [512,256]
                n_ch = (gc + 511) // 512
                base_w, extra = divmod(gc, n_ch)
                chunks = []
                c0 = 0
                for ci in range(n_ch):
                    cw = base_w + (1 if ci < extra else 0)
                    chunks.append((c0, cw))
                    c0 += cw

                # ---- stage 1: hT[f, c] = silu(wg.T x) * (wu.T x) ----
                ht_tiles = []
                for ft in range(FT):
                    if ft in wgu_pre:
                        wgt, wut = wgu_pre.pop(ft)
                    else:
                        wgt = wgu_pool.tile([P, DT, P], f32r, name="wgt")
                        nc.sync.dma_start(
                            out=wgt[:], in_=wg_r[:, :, ft * P : (ft + 1) * P]
                        )
                        wut = wgu_pool.tile([P, DT, P], f32r, name="wut")
                        nc.sync.dma_start(
                            out=wut[:], in_=wu_r[:, :, ft * P : (ft + 1) * P]
                        )
                    ht = ht_pool.tile([P, G_MAX], f32r, name="ht")
                    ht_tiles.append(ht)
                    for c0, cw in chunks:
                        psg = ps1_pool.tile([P, 512], fp32, name="psg")
                        for dt_i in range(DT):
                            nc.tensor.matmul(
                                psg[:, :cw],
                                wgt[:, dt_i, :],
                                xg[:, dt_i, c0 : c0 + cw],
                                start=(dt_i == 0),
                                stop=(dt_i == DT - 1),
                            )
                        psu = ps1_pool.tile([P, 512], fp32, name="psu")
                        for dt_i in range(DT):
                            nc.tensor.matmul(
                                psu[:, :cw],
                                wut[:, dt_i, :],
                                xg[:, dt_i, c0 : c0 + cw],
                                start=(dt_i == 0),
                                stop=(dt_i == DT - 1),
                            )
                        sil = act_pool.tile([P, 512], fp32, name="sil")
                        nc.scalar.activation(sil[:, :cw], psg[:, :cw], silu_fn)
                        nc.vector.tensor_tensor(
                            out=ht[:, c0 : c0 + cw],
                            in0=sil[:, :cw],
                            in1=psu[:, :cw],
                            op=mult_op,
                        )

                # ---- stage 2: y[c, d] = sum_f hT[f, c] * wd[f, d], scaled ----
                # c_subs in batches of <=4 so stage-2 PSUM stays in 6 banks.
                for d0 in range(0, D_MODEL, 512):
                    for bi, cs_list in enumerate(batches):
                        ps_out = [
                            ps2_pool.tile([P, 512], fp32, name="pso") for _ in cs_list
                        ]
                        for ft0 in range(0, FT, 2):
                            # two f-tiles of wd per DMA: 512KB transfers, each
                            # feeding 2*len(cs_list) matmuls
                            wdt = wd_pre.pop((d0, bi, ft0), None)
                            if wdt is None:
                                wdt = wd_pool.tile([P, 2, 512], f32r, name="wdt")
                                nc.scalar.dma_start(
                                    out=wdt[:],
                                    in_=wd_r[:, ft0 : ft0 + 2, d0 : d0 + 512],
                                )
                            for fi in range(2):
                                ft = ft0 + fi
                                for i, cs in enumerate(cs_list):
                                    nc.tensor.matmul(
                                        ps_out[i][:],
                                        ht_tiles[ft][:, cs * P : (cs + 1) * P],
                                        wdt[:, fi, :],
                                        start=(ft == 0),
                                        stop=(ft == FT - 1),
                                    )
                        for i, cs in enumerate(cs_list):
                            ot = out_pool.tile([P, 512], fp32, name="ot")
                            if i % 2 == 0:
                                nc.vector.tensor_scalar_mul(
                                    ot[:], ps_out[i][:], sct[:, cs : cs + 1]
                                )
                            else:
                                # spread evictions across engines so the bank
                                # ring frees faster at d0 boundaries
                                nc.scalar.activation(
                                    ot[:],
                                    ps_out[i][:],
                                    mybir.ActivationFunctionType.Copy,
                                    scale=sct[:, cs : cs + 1],
                                )
                            r0 = g0 + cs * P
                            nc.gpsimd.dma_start(
                                out=y_ap[r0 : r0 + P, d0 : d0 + 512], in_=ot[:]
                            )
    nc.compile()
    return nc


def _get_program(C):
    if C not in _programs:
        _programs[C] = _build_program(C)
    return _programs[C]


def _route(xf, router_w):
    """Host router, float64 (all f32 evaluation orders agree on this input's
    top-2 sets; f64 is the stable reference ranking). Mirrors
    softmax -> top_k(2) -> renormalize from the reference."""
    logits = xf.astype(np.float64) @ router_w.astype(np.float64).T
    logits -= logits.max(axis=-1, keepdims=True)
    sm = np.exp(logits)
    sm /= sm.sum(axis=-1, keepdims=True)
    top = np.argsort(-sm, axis=-1, kind="stable")[:, :TOP_K]
    tsc = np.take_along_axis(sm, top, axis=1)
    tsc = tsc / tsc.sum(axis=-1, keepdims=True)
    return top, tsc


def kernel(x, router_w, w_gate, w_up, w_down):
    global LAST_EXEC_NS
    from concourse.bass_utils import run_bass_kernel_spmd

    trace = os.environ.get("MOE_TRACE", "0") == "1"
    if trace:
        _ensure_axon_hooks()

    x = np.asarray(x, dtype=np.float32)
    router_w = np.asarray(router_w, dtype=np.float32)
    w_gate = np.ascontiguousarray(np.asarray(w_gate, dtype=np.float32))
    w_up = np.ascontiguousarray(np.asarray(w_up, dtype=np.float32))
    w_down = np.ascontiguousarray(np.asarray(w_down, dtype=np.float32))

    B, T, D = x.shape
    N = B * T
    xf = np.ascontiguousarray(x.reshape(N, D))

    top, tsc = _route(xf, router_w)

    tok_rows = []
    tok_wts = []
    for e in range(N_EXPERTS):
        mask = top == e
        rows = np.nonzero(mask.any(axis=1))[0]
        wts = tsc[mask].astype(np.float32)
        tok_rows.append(rows)
        tok_wts.append(wts)

    cmax = max(max(len(r) for r in tok_rows), 1)
    C = max(((cmax + P - 1) // P) * P, 256)

    nc = _get_program(C)

    in_maps = []
    for e in range(N_EXPERTS):
        rows = tok_rows[e]
        xg = np.zeros((C, D), np.float32)
        xg[: len(rows)] = xf[rows]
        scv = np.zeros((C,), np.float32)
        scv[: len(rows)] = tok_wts[e]
        in_maps.append(
            {
                "xT": np.ascontiguousarray(xg.T),
                "wg": w_gate[e],
                "wu": w_up[e],
                "wd": w_down[e],
                "sc": scv,
            }
        )

    res = run_bass_kernel_spmd(nc, in_maps, list(range(N_EXPERTS)), trace=trace)
    if trace:
        LAST_EXEC_NS = res.exec_time_ns

    out = np.zeros((N, D), np.float32)
    for e in range(N_EXPERTS):
        rows = tok_rows[e]
        out[rows] += res.results[e]["y"][: len(rows)]
    return out.reshape(B, T, D)



# revision 4
# speedup vs baseline: 1.0925x; 1.0925x over previous
"""MoE layer (SwiGLU experts, top-2 routing) on 8 Trainium2 NeuronCores.

Strategy (expert parallelism, per the sharding hint):
  - The router (a [N,8] matmul + softmax + top-2, ~0.01% of total FLOPs) is
    computed host-side in float64; it determines the token->expert dispatch.
  - Token dispatch/combine (the "all-to-all") is done host-side: each core e
    receives expert e's weights plus the tokens routed to expert e, padded to
    a uniform capacity C (multiple of 128, same on all cores for SPMD).
  - Each core runs the heavy compute in bf16 (full PE rate, rel-err ~4e-3
    which is well under the 2e-2 budget); accumulation stays fp32 in PSUM.
  - The per-token combine weight is folded into a second, pre-scaled copy of
    the token slab that feeds the up-projection: silu(x@wg) * ((s*x)@wu)
    equals s * (silu(x@wg) * (x@wu)), so stage 2 needs no scaling at all.
  - Host scatter-adds each expert's (transposed) output back into the output.

Device kernel structure (per core):
  Tokens are processed in groups of ~1024 (512-wide matmul chunks). Stage 1
  computes hT[f, c] = silu(wg.T x) * (wu.T xs) for all F=4096 rows of the
  group, accumulating over D=1024 in PSUM, gate/up banks drained by ScalarE
  (silu) and VectorE (mul, cast to bf16) into SBUF. Stage 2 is transposed:
  stationary = wd tile [128f, 128d], moving = hT[f, c-span], accumulating
  yT[d, c] over F in PSUM (32 steps), so wd streams exactly once per group
  and consecutive matmuls share the stationary weights. Output is written
  transposed (yT [D, C]); the host transposes during the combine.
"""

import os
import sys

sys.path.insert(0, "/opt/trn_rl_repo")
import numpy as np

P = 128
D_MODEL = 1024
D_FF = 4096
N_EXPERTS = 8
TOP_K = 2

LAST_EXEC_NS = None
_programs = {}


def _ensure_axon_hooks():
    """The agent image's antenv lacks axon_hooks; reconstruct it so
    trace=True works (NTFF profiling via libaxon_pjrt ctypes hook)."""
    import types

    try:
        import antenv.axon_hooks  # noqa: F401

        return
    except ImportError:
        pass
    try:
        import antenv

        mod = types.ModuleType("antenv.axon_hooks")
        _hook = [None]
        mod.set_axon_ntff_profile_hook = lambda h: _hook.__setitem__(0, h)
        mod.get_axon_ntff_profile_hook = lambda: _hook[0]
        sys.modules["antenv.axon_hooks"] = mod
        antenv.axon_hooks = mod
        if "/root/.axon_site" not in sys.path:
            sys.path.insert(0, "/root/.axon_site")
        from trn_agent_boot.trn_boot import _ntff_profile_via_ctypes

        mod.set_axon_ntff_profile_hook(
            _ntff_profile_via_ctypes("/opt/axon/libaxon_pjrt.so")
        )
        import concourse.bass_utils as bu

        bu.upload_artifacts = lambda tmpdir: f"local://{tmpdir}"
    except Exception:
        pass


def _group_plan(C):
    """Split C tokens into groups (<=1536), preferring 1024 so chunks are
    512-wide; fold a short remainder into the last group."""
    k, r = divmod(C, 1024)
    if k == 0:
        return [C]
    if r == 0:
        return [1024] * k
    if r >= 512:
        return [1024] * k + [r]
    return [1024] * (k - 1) + [1024 + r]


def _chunk_plan(gc):
    """512-wide chunks with one (possibly narrow) tail chunk."""
    spans = []
    c0 = 0
    while c0 < gc:
        w = min(512, gc - c0)
        spans.append((c0, w))
        c0 += w
    return spans


def _build_program(C):
    import concourse.bacc as bacc
    import concourse.mybir as mybir
    from concourse.tile import TileContext

    fp32 = mybir.dt.float32
    bf16 = mybir.dt.bfloat16
    D, F = D_MODEL, D_FF
    DT, FT = D // P, F // P
    silu_fn = mybir.ActivationFunctionType.Silu
    mult_op = mybir.AluOpType.mult

    nc = bacc.Bacc(
        "TRN2", target_bir_lowering=False, debug=False, num_devices=N_EXPERTS
    )
    xT = nc.dram_tensor("xT", [D, C], bf16, kind="ExternalInput")
    xuT = nc.dram_tensor("xuT", [D, C], bf16, kind="ExternalInput")
    wg = nc.dram_tensor("wg", [D, F], bf16, kind="ExternalInput")
    wu = nc.dram_tensor("wu", [D, F], bf16, kind="ExternalInput")
    wd = nc.dram_tensor("wd", [F, D], bf16, kind="ExternalInput")
    yT = nc.dram_tensor("yT", [D, C], fp32, kind="ExternalOutput")

    xT_r = xT.ap().rearrange("(dt p) c -> p dt c", p=P)
    xuT_r = xuT.ap().rearrange("(dt p) c -> p dt c", p=P)
    wg_r = wg.ap().rearrange("(dt p) f -> p dt f", p=P)
    wu_r = wu.ap().rearrange("(dt p) f -> p dt f", p=P)
    wd_r = wd.ap().rearrange("(ft p) d -> p ft d", p=P)
    yT_ap = yT.ap()

    sizes = _group_plan(C)
    groups = []
    g0 = 0
    for gc in sizes:
        groups.append((g0, gc))
        g0 += gc
    gmax = max(sizes)

    with TileContext(nc) as tc:
        with (
            tc.tile_pool(name="warm", bufs=1) as warm_pool,
            tc.tile_pool(name="xg", bufs=2) as xg_pool,
            tc.tile_pool(name="xu", bufs=2) as xu_pool,
            tc.tile_pool(name="wgu", bufs=4) as wgu_pool,
            tc.tile_pool(name="ht", bufs=FT + 4) as ht_pool,
            tc.tile_pool(name="wdp", bufs=6) as wd_pool,
            tc.tile_pool(name="act", bufs=2) as act_pool,
            tc.tile_pool(name="out", bufs=4) as out_pool,
            tc.tile_pool(name="ps1", bufs=1, space="PSUM") as ps1_pool,
            tc.tile_pool(name="ps2", bufs=6, space="PSUM") as ps2_pool,
        ):
            # Warm-up: keep TensorE busy while the first tiles stream in, so
            # the HAM clock gate reaches 2.4 GHz before real matmuls start.
            wsrc = warm_pool.tile([P, 512], bf16, name="wsrc")
            nc.vector.memset(wsrc.bitcast(fp32)[:], 0.0)
            wps = ps1_pool.tile([P, 512], fp32, name="psg")
            for wi in range(60):
                nc.tensor.matmul(
                    wps[:, :256],
                    wsrc[:, :P],
                    wsrc[:, :256],
                    start=(wi == 0),
                    stop=(wi == 59),
                )
            for g0, gc in groups:
                spans = _chunk_plan(gc)

                # hoist the first f-tile's weights ahead of the token slab so
                # stage 1 can start as soon as xg's first slice lands
                wgu_pre = {}
                for ft in range(1):
                    wgt = wgu_pool.tile([P, DT, P], bf16, name="wgt")
                    nc.sync.dma_start(
                        out=wgt[:], in_=wg_r[:, :, ft * P : (ft + 1) * P]
                    )
                    wut = wgu_pool.tile([P, DT, P], bf16, name="wut")
                    nc.sync.dma_start(
                        out=wut[:], in_=wu_r[:, :, ft * P : (ft + 1) * P]
                    )
                    wgu_pre[ft] = (wgt, wut)

                xg = xg_pool.tile([P, DT, gmax], bf16, name="xg")
                xu = xu_pool.tile([P, DT, gmax], bf16, name="xu")
                # per-dt loads so stage-1 matmuls can start on the first slice
                for dt_i in range(DT):
                    nc.sync.dma_start(
                        out=xg[:, dt_i, :gc], in_=xT_r[:, dt_i, g0 : g0 + gc]
                    )
                    nc.sync.dma_start(
                        out=xu[:, dt_i, :gc], in_=xuT_r[:, dt_i, g0 : g0 + gc]
                    )

                # ---- stage 1: hT[f, c] = silu(wg.T x) * (wu.T xs) ----
                ht_tiles = []
                for ft in range(FT):
                    if ft in wgu_pre:
                        wgt, wut = wgu_pre.pop(ft)
                    else:
                        wgt = wgu_pool.tile([P, DT, P], bf16, name="wgt")
                        nc.sync.dma_start(
                            out=wgt[:], in_=wg_r[:, :, ft * P : (ft + 1) * P]
                        )
                        wut = wgu_pool.tile([P, DT, P], bf16, name="wut")
                        nc.sync.dma_start(
                            out=wut[:], in_=wu_r[:, :, ft * P : (ft + 1) * P]
                        )
                    ht = ht_pool.tile([P, gmax], bf16, name="ht")
                    ht_tiles.append(ht)
                    for c0, cw in spans:
                        psg = ps1_pool.tile([P, 512], fp32, name="psg")
                        for dt_i in range(DT):
                            nc.tensor.matmul(
                                psg[:, :cw],
                                wgt[:, dt_i, :],
                                xg[:, dt_i, c0 : c0 + cw],
                                start=(dt_i == 0),
                                stop=(dt_i == DT - 1),
                            )
                        psu = ps1_pool.tile([P, 512], fp32, name="psu")
                        for dt_i in range(DT):
                            nc.tensor.matmul(
                                psu[:, :cw],
                                wut[:, dt_i, :],
                                xu[:, dt_i, c0 : c0 + cw],
                                start=(dt_i == 0),
                                stop=(dt_i == DT - 1),
                            )
                        sil = act_pool.tile([P, 512], fp32, name="sil")
                        nc.scalar.activation(sil[:, :cw], psg[:, :cw], silu_fn)
                        nc.vector.tensor_tensor(
                            out=ht[:, c0 : c0 + cw],
                            in0=sil[:, :cw],
                            in1=psu[:, :cw],
                            op=mult_op,
                        )

                # ---- stage 2 (transposed): yT[d, c] = sum_f wd[f, d] hT[f, c]
                # stationary = wd tile [128f, 128d], moving = hT span; wd
                # streams exactly once per group; spans share the stationary.
                FTB = 8  # f-tiles per wd DMA (2KB per partition)
                for db in range(D // P):
                    ps_out = [
                        ps2_pool.tile([P, 512], fp32, name="pso") for _ in spans
                    ]
                    for ftb in range(FT // FTB):
                        wdt = wd_pool.tile([P, FTB, P], bf16, name="wdt")
                        nc.scalar.dma_start(
                            out=wdt[:],
                            in_=wd_r[:, ftb * FTB : (ftb + 1) * FTB, db * P : (db + 1) * P],
                        )
                        for fi in range(FTB):
                            ft = ftb * FTB + fi
                            for si, (c0, cw) in enumerate(spans):
                                nc.tensor.matmul(
                                    ps_out[si][:, :cw],
                                    wdt[:, fi, :],
                                    ht_tiles[ft][:, c0 : c0 + cw],
                                    start=(ft == 0),
                                    stop=(ft == FT - 1),
                                )
                    for si, (c0, cw) in enumerate(spans):
                        ot = out_pool.tile([P, 512], fp32, name="ot")
                        if si % 2 == 0:
                            nc.vector.tensor_scalar_mul(
                                ot[:, :cw], ps_out[si][:, :cw], 1.0
                            )
                        else:
                            # spread evictions across engines so the bank
                            # ring frees faster at d-tile boundaries
                            nc.scalar.activation(
                                ot[:, :cw],
                                ps_out[si][:, :cw],
                                mybir.ActivationFunctionType.Copy,
                            )
                        nc.gpsimd.dma_start(
                            out=yT_ap[db * P : (db + 1) * P, g0 + c0 : g0 + c0 + cw],
                            in_=ot[:, :cw],
                        )
    nc.compile()
    return nc


def _get_program(C):
    if C not in _programs:
        _programs[C] = _build_program(C)
    return _programs[C]


def _route(xf, router_w):
    """Host router, float64 (all f32 evaluation orders agree on this input's
    top-2 sets; f64 is the stable reference ranking). Mirrors
    softmax -> top_k(2) -> renormalize from the reference."""
    logits = xf.astype(np.float64) @ router_w.astype(np.float64).T
    logits -= logits.max(axis=-1, keepdims=True)
    sm = np.exp(logits)
    sm /= sm.sum(axis=-1, keepdims=True)
    top = np.argsort(-sm, axis=-1, kind="stable")[:, :TOP_K]
    tsc = np.take_along_axis(sm, top, axis=1)
    tsc = tsc / tsc.sum(axis=-1, keepdims=True)
    return top, tsc


def kernel(x, router_w, w_gate, w_up, w_down):
    global LAST_EXEC_NS
    import ml_dtypes
    from concourse.bass_utils import run_bass_kernel_spmd

    bf = ml_dtypes.bfloat16

    trace = os.environ.get("MOE_TRACE", "0") == "1"
    if trace:
        _ensure_axon_hooks()

    x = np.asarray(x, dtype=np.float32)
    router_w = np.asarray(router_w, dtype=np.float32)

    B, T, D = x.shape
    N = B * T
    xf = np.ascontiguousarray(x.reshape(N, D))

    top, tsc = _route(xf, router_w)

    tok_rows = []
    tok_wts = []
    for e in range(N_EXPERTS):
        mask = top == e
        rows = np.nonzero(mask.any(axis=1))[0]
        wts = tsc[mask].astype(np.float32)
        tok_rows.append(rows)
        tok_wts.append(wts)

    cmax = max(max(len(r) for r in tok_rows), 1)
    C = max(((cmax + P - 1) // P) * P, 256)

    nc = _get_program(C)

    in_maps = []
    for e in range(N_EXPERTS):
        rows = tok_rows[e]
        xg = np.zeros((C, D), np.float32)
        xg[: len(rows)] = xf[rows]
        xs = np.zeros((C, D), np.float32)
        xs[: len(rows)] = xf[rows] * tok_wts[e][:, None]
        in_maps.append(
            {
                "xT": np.ascontiguousarray(xg.T.astype(bf)),
                "xuT": np.ascontiguousarray(xs.T.astype(bf)),
                "wg": np.ascontiguousarray(np.asarray(w_gate[e], np.float32).astype(bf)),
                "wu": np.ascontiguousarray(np.asarray(w_up[e], np.float32).astype(bf)),
                "wd": np.ascontiguousarray(np.asarray(w_down[e], np.float32).astype(bf)),
            }
        )

    res = run_bass_kernel_spmd(nc, in_maps, list(range(N_EXPERTS)), trace=trace)
    if trace:
        LAST_EXEC_NS = res.exec_time_ns

    out = np.zeros((N, D), np.float32)
    for e in range(N_EXPERTS):
        rows = tok_rows[e]
        out[rows] += res.results[e]["yT"][:, : len(rows)].T
    return out.reshape(B, T, D)
